# revision 18
# baseline (speedup 1.0000x reference)
"""Trainium2 Bass kernel for nn_MDN_MultivariateNormal (B=1024, T=30, M=20).

Pure data-parallel over batch: 8 NeuronCores x 128 batch rows each.
Batch rows map to the 128 SBUF partitions; everything else lives in the
free dimension, so all compute is single-pass elementwise/reduction work.

Outputs (per core shard, 128 rows):
  probs_traj [128,20], loc [128,20,60], chol_full [128,20,60,60],
  step_probs [128,30,20], mu [128,30,20,2], chol_step [128,30,20,2,2]

chol_full is 2x2-block-diagonal: nonzeros at flat offsets 122t (L11),
122t+60 (L21), 122t+61 (L22) within each 3600-elem [60,60] block.
We keep rotating SBUF buffers that are zeroed once, write only the three
strided diagonals per (m), and DMA the dense 14.4KB/partition block out.
"""

import sys

for _p in ("/root/.axon_site/_ro/trn_rl_repo", "/opt/trn_rl_repo"):
    if _p not in sys.path:
        sys.path.append(_p)

from contextlib import ExitStack

import numpy as np

import concourse.bacc as bacc
import concourse.bass as bass
import concourse.mybir as mybir
import concourse.tile as tile
from concourse.bass_utils import run_bass_kernel_spmd

F32 = mybir.dt.float32
AF = mybir.ActivationFunctionType
ALU = mybir.AluOpType
AX = mybir.AxisListType

B, T, M = 1024, 30, 20
NCORES = 8
BC = B // NCORES  # 128 batch rows per core == SBUF partitions
D = 2 * T  # 60
BLK = D * D  # 3600

# tunables
NBUF = 4  # rotating chol_full SBUF buffers (dense variant)
STEP_BCAST = True  # use stride-0 broadcast tensor_tensor for step softmax
SPARSE_CHOL = True  # rely on pre-zeroed DRAM outputs: write only the
# three block-diagonals of chol_full as strided DMA runs instead of
# streaming the dense (mostly zero) 295 MB


def _diag_ap(base, extra_offset, step, count):
    """Strided free-dim AP into an SBUF tile: [128p][count elems, stride step]."""
    return bass.AP(
        tensor=base.tensor,
        offset=base.offset + extra_offset,
        ap=[list(base.ap[0]), [step, count]],
    )


def _build_body(ctx, tc, ins, outs):
    nc = tc.nc
    ml, ptw, psw = ins
    probs_traj, loc, chol_full, step_probs, mu, chol_step = outs

    pool = ctx.enter_context(tc.tile_pool(name="main", bufs=1))
    cpool = ctx.enter_context(tc.tile_pool(name="cf", bufs=1))

    # ---- load inputs (SP HWDGE ring; these precede the chol stream) ----
    X = pool.tile([BC, T, M, 5], F32, tag="X")
    nc.sync.dma_start(out=X[:], in_=ml.ap().rearrange("p (t m f) -> p t m f", t=T, m=M))
    PT = pool.tile([BC, M], F32, tag="PT")
    nc.sync.dma_start(out=PT[:], in_=ptw.ap())
    PS = pool.tile([BC, T, M], F32, tag="PS")
    nc.sync.dma_start(out=PS[:], in_=psw.ap().rearrange("p (t m) -> p t m", t=T))

    # ---- pre-zero the rotating chol buffers (zeros persist; only diagonals
    # are rewritten per m, so zero once per buffer) ----
    if not SPARSE_CHOL:
        cbufs = [
            cpool.tile([BC, BLK], F32, tag=f"cf{i}", name=f"cf{i}")
            for i in range(NBUF)
        ]
        for cb in cbufs:
            nc.vector.memset(cb[:], 0.0)

    # constant bias tiles for activation(func(scale*x + bias))
    bias_one = pool.tile([BC, 1], F32, tag="bias_one")
    nc.vector.memset(bias_one, 1.0)
    bias_gate = pool.tile([BC, 1], F32, tag="bias_gate")
    nc.vector.memset(bias_gate, -0.2)

    # ---- per-component Cholesky params, in [BC, T, M] (input) layout ----
    SX = pool.tile([BC, T, M], F32, tag="SX")  # sqrt(exp(p2)+eps) == L11
    nc.scalar.activation(out=SX[:], in_=X[:, :, :, 2], func=AF.Exp)
    nc.vector.tensor_scalar_add(out=SX[:], in0=SX[:], scalar1=1e-6)
    nc.scalar.activation(out=SX[:], in_=SX[:], func=AF.Sqrt)

    SY = pool.tile([BC, T, M], F32, tag="SY")
    nc.scalar.activation(out=SY[:], in_=X[:, :, :, 3], func=AF.Exp)
    nc.vector.tensor_scalar_add(out=SY[:], in0=SY[:], scalar1=1e-6)
    nc.scalar.activation(out=SY[:], in_=SY[:], func=AF.Sqrt)

    RHO = pool.tile([BC, T, M], F32, tag="RHO")
    nc.scalar.activation(out=RHO[:], in_=X[:, :, :, 4], func=AF.Tanh)

    L21 = pool.tile([BC, T, M], F32, tag="L21")
    nc.vector.tensor_mul(out=L21[:], in0=RHO[:], in1=SY[:])

    L22 = pool.tile([BC, T, M], F32, tag="L22")
    nc.vector.tensor_mul(out=L22[:], in0=RHO[:], in1=RHO[:])
    nc.scalar.activation(out=L22[:], in_=L22[:], func=AF.Sqrt, scale=-1.0, bias=bias_one)
    nc.vector.tensor_mul(out=L22[:], in0=L22[:], in1=SY[:])

    # ---- chol_full ----
    cf = chol_full.ap().rearrange("p (m k) -> p m k", m=M)
    if SPARSE_CHOL:
        # DRAM outputs arrive pre-zeroed (native run_bass_kernel_spmd zeros
        # them; the PJRT path donates zero buffers), so only the nonzero
        # diagonals need writing: per (m, t) the L11 scalar at 122t and the
        # [L21, L22] pair at 122t+60. DMA APs max out at 3 dims with a
        # contiguous last dim, so issue one DMA per (m, diagonal-kind).
        SL = pool.tile([BC, M, T, 2], F32, tag="SL")
        nc.vector.tensor_copy(out=SL[:, :, :, 0], in_=L21[:].transpose([0, 2, 1]))
        nc.vector.tensor_copy(out=SL[:, :, :, 1], in_=L22[:].transpose([0, 2, 1]))
        cfb = cf[:, 0, 0:1]  # AP anchor for manual strided APs
        pdim = list(cfb.ap[0])
        for m in range(M):
            l11_out = bass.AP(
                tensor=cfb.tensor,
                offset=cfb.offset + m * BLK,
                ap=[pdim, [2 * D + 2, T], [1, 1]],
            )
            nc.scalar.dma_start(
                out=l11_out, in_=SX[:, :, m].unsqueeze(2)
            )
            l2122_out = bass.AP(
                tensor=cfb.tensor,
                offset=cfb.offset + m * BLK + D,
                ap=[pdim, [2 * D + 2, T], [1, 2]],
            )
            nc.sync.dma_start(out=l2122_out, in_=SL[:, m, :, :])
    else:
        for m in range(M):
            cb = cbufs[m % NBUF]
            base = cb[:]
            # tensor_scalar_add (not tensor_copy): the DVE TensorCopy "TR"
            # encoding has a single sync-wait slot and walrus dies when Tile
            # needs two waits (producer RAW + DMA WAR) on one instruction.
            nc.vector.tensor_scalar_add(
                out=_diag_ap(base, 0, 2 * D + 2, T), in0=SX[:, :, m], scalar1=0.0
            )
            nc.vector.tensor_scalar_add(
                out=_diag_ap(base, D, 2 * D + 2, T), in0=L21[:, :, m], scalar1=0.0
            )
            nc.vector.tensor_scalar_add(
                out=_diag_ap(base, D + 1, 2 * D + 2, T), in0=L22[:, :, m], scalar1=0.0
            )
            nc.sync.dma_start(out=cf[:, m, :], in_=base)

    # ---- loc [BC, M, T, 2] and mu [BC, T, M, 2] ----
    LOC = pool.tile([BC, M, T, 2], F32, tag="LOC")
    nc.vector.tensor_copy(out=LOC[:, :, :, 0], in_=X[:, :, :, 0].transpose([0, 2, 1]))
    nc.vector.tensor_copy(out=LOC[:, :, :, 1], in_=X[:, :, :, 1].transpose([0, 2, 1]))
    nc.scalar.dma_start(out=loc.ap(), in_=LOC[:].rearrange("p a b c -> p (a b c)"))

    MUT = pool.tile([BC, T, M, 2], F32, tag="MUT")
    nc.vector.tensor_copy(out=MUT[:, :, :, 0], in_=X[:, :, :, 0])
    nc.vector.tensor_copy(out=MUT[:, :, :, 1], in_=X[:, :, :, 1])
    nc.scalar.dma_start(out=mu.ap(), in_=MUT[:].rearrange("p a b c -> p (a b c)"))

    # ---- chol_step [BC, T, M, 2, 2] = [[L11,0],[L21,L22]] ----
    CS = pool.tile([BC, T, M, 4], F32, tag="CS")
    nc.vector.memset(CS[:, :, :, 1], 0.0)
    nc.vector.tensor_copy(out=CS[:, :, :, 0], in_=SX[:])
    nc.vector.tensor_copy(out=CS[:, :, :, 2], in_=L21[:])
    nc.vector.tensor_copy(out=CS[:, :, :, 3], in_=L22[:])
    nc.scalar.dma_start(out=chol_step.ap(), in_=CS[:].rearrange("p a b c -> p (a b c)"))

    # ---- probs_traj: pruned softmax over M=20 ----
    E1 = pool.tile([BC, M], F32, tag="E1")
    nc.scalar.activation(out=E1[:], in_=PT[:], func=AF.Exp)
    s1 = pool.tile([BC, 1], F32, tag="s1")
    nc.vector.reduce_sum(out=s1[:], in_=E1[:], axis=AX.X)
    nc.vector.reciprocal(out=s1[:], in_=s1[:])
    nc.vector.tensor_scalar_mul(out=E1[:], in0=E1[:], scalar1=s1[:])  # p
    G1 = pool.tile([BC, M], F32, tag="G1")
    nc.scalar.activation(out=G1[:], in_=E1[:], func=AF.Sigmoid, scale=10.0, bias=bias_gate)
    nc.vector.tensor_mul(out=E1[:], in0=E1[:], in1=G1[:])  # p_tilde
    s2 = pool.tile([BC, 1], F32, tag="s2")
    nc.vector.reduce_sum(out=s2[:], in_=E1[:], axis=AX.X)
    nc.vector.tensor_scalar_add(out=s2[:], in0=s2[:], scalar1=1e-8)
    nc.vector.reciprocal(out=s2[:], in_=s2[:])
    nc.vector.tensor_scalar(
        out=E1[:], in0=E1[:], scalar1=s2[:], scalar2=1e-8, op0=ALU.mult, op1=ALU.add
    )  # p_new + eps
    s3 = pool.tile([BC, 1], F32, tag="s3")
    nc.vector.reduce_sum(out=s3[:], in_=E1[:], axis=AX.X)
    nc.vector.reciprocal(out=s3[:], in_=s3[:])
    nc.vector.tensor_scalar_mul(out=E1[:], in0=E1[:], scalar1=s3[:])
    nc.scalar.dma_start(out=probs_traj.ap(), in_=E1[:])

    # ---- step_probs: pruned softmax over M per (row, t) ----
    E2 = pool.tile([BC, T, M], F32, tag="E2")
    nc.scalar.activation(out=E2[:], in_=PS[:], func=AF.Exp)
    t1 = pool.tile([BC, T], F32, tag="t1")
    nc.vector.reduce_sum(out=t1[:], in_=E2[:], axis=AX.X)
    nc.vector.reciprocal(out=t1[:], in_=t1[:])

    def bmul(dst, src, vec):
        """dst[p,t,m] = src[p,t,m] * vec[p,t] (broadcast over m)."""
        if STEP_BCAST:
            nc.vector.tensor_tensor(
                out=dst, in0=src, in1=vec.broadcast_to([BC, T, M]), op=ALU.mult
            )
        else:
            for t in range(T):
                nc.vector.tensor_scalar_mul(
                    out=dst[:, t, :], in0=src[:, t, :], scalar1=vec[:, t : t + 1]
                )

    bmul(E2[:], E2[:], t1[:])  # p
    G2 = pool.tile([BC, T, M], F32, tag="G2")
    nc.scalar.activation(out=G2[:], in_=E2[:], func=AF.Sigmoid, scale=10.0, bias=bias_gate)
    nc.vector.tensor_mul(out=E2[:], in0=E2[:], in1=G2[:])  # p_tilde
    t2 = pool.tile([BC, T], F32, tag="t2")
    nc.vector.reduce_sum(out=t2[:], in_=E2[:], axis=AX.X)
    nc.vector.tensor_scalar_add(out=t2[:], in0=t2[:], scalar1=1e-8)
    nc.vector.reciprocal(out=t2[:], in_=t2[:])
    bmul(E2[:], E2[:], t2[:])
    nc.vector.tensor_scalar_add(out=E2[:], in0=E2[:], scalar1=1e-8)  # p_new + eps
    t3 = pool.tile([BC, T], F32, tag="t3")
    nc.vector.reduce_sum(out=t3[:], in_=E2[:], axis=AX.X)
    nc.vector.reciprocal(out=t3[:], in_=t3[:])
    bmul(E2[:], E2[:], t3[:])
    nc.scalar.dma_start(
        out=step_probs.ap(), in_=E2[:].rearrange("p a b -> p (a b)")
    )


def build_nc():
    # Bacc (not plain Bass): its compile() pass splits multi-wait sync_info
    # into EventSemaphore instructions -- TRN2 compute instructions accept
    # only one embedded wait and walrus rejects more.
    nc = bacc.Bacc("TRN2", target_bir_lowering=False, debug=False)
    ml = nc.dram_tensor("mixture_latent", [BC, T * M * 5], F32, kind="ExternalInput")
    ptw = nc.dram_tensor("per_traj_weight", [BC, M], F32, kind="ExternalInput")
    psw = nc.dram_tensor("per_step_weight", [BC, T * M], F32, kind="ExternalInput")
    o_probs = nc.dram_tensor("probs_traj", [BC, M], F32, kind="ExternalOutput")
    o_loc = nc.dram_tensor("loc", [BC, M * D], F32, kind="ExternalOutput")
    o_chol = nc.dram_tensor("chol_full", [BC, M * BLK], F32, kind="ExternalOutput")
    o_step = nc.dram_tensor("step_probs", [BC, T * M], F32, kind="ExternalOutput")
    o_mu = nc.dram_tensor("mu", [BC, T * M * 2], F32, kind="ExternalOutput")
    o_cs = nc.dram_tensor("chol_step", [BC, T * M * 4], F32, kind="ExternalOutput")
    with tile.TileContext(nc) as tc:
        with ExitStack() as ctx:
            _build_body(
                ctx,
                tc,
                (ml, ptw, psw),
                (o_probs, o_loc, o_chol, o_step, o_mu, o_cs),
            )
    nc.compile()
    return nc


_NC_CACHE = None


def _get_nc():
    global _NC_CACHE
    if _NC_CACHE is None:
        _NC_CACHE = build_nc()
    return _NC_CACHE


_RUNNER_CACHE = None


def _get_runner():
    """Build the jitted 8-core executable once and reuse it across calls.

    Mirrors concourse.bass2jax.run_bass_via_pjrt's multi-core path, but
    caches the jax.jit(shard_map(...)) callable so repeat kernel() calls
    skip re-lowering/XLA compile.
    """
    global _RUNNER_CACHE
    if _RUNNER_CACHE is not None:
        return _RUNNER_CACHE

    import jax
    from jax.experimental.shard_map import shard_map
    from jax.sharding import Mesh, PartitionSpec

    from concourse import bass2jax as b2j

    nc = _get_nc()
    b2j.install_neuronx_cc_hook()

    partition_name = (
        nc.partition_id_tensor.name if nc.partition_id_tensor else None
    )
    in_names: list[str] = []
    out_names: list[str] = []
    out_avals: list = []
    zero_shapes: list = []
    for alloc in nc.m.functions[0].allocations:
        if not isinstance(alloc, mybir.MemoryLocationSet):
            continue
        name = alloc.memorylocations[0].name
        if alloc.kind == "ExternalInput":
            if name != partition_name:
                in_names.append(name)
        elif alloc.kind == "ExternalOutput":
            shape = tuple(alloc.tensor_shape)
            dtype = mybir.dt.np(alloc.dtype)
            out_avals.append(jax.core.ShapedArray(shape, dtype))
            out_names.append(name)
            zero_shapes.append((shape, dtype))
    n_params = len(in_names)
    n_outs = len(out_names)
    all_names = in_names + out_names
    if partition_name is not None:
        all_names = all_names + [partition_name]

    def _body(*args):
        operands = list(args)
        if partition_name is not None:
            operands.append(b2j.partition_id_tensor())
        outs = b2j._bass_exec_p.bind(
            *operands,
            out_avals=tuple(out_avals),
            in_names=tuple(all_names),
            out_names=tuple(out_names),
            lowering_input_output_aliases=(),
            sim_require_finite=True,
            sim_require_nnan=True,
            nc=nc,
        )
        return tuple(outs)

    devices = jax.devices()[:NCORES]
    mesh = Mesh(np.asarray(devices), ("core",))
    donate = tuple(range(n_params, n_params + n_outs))
    sharded = jax.jit(
        shard_map(
            _body,
            mesh=mesh,
            in_specs=(PartitionSpec("core"),) * (n_params + n_outs),
            out_specs=(PartitionSpec("core"),) * n_outs,
            check_rep=False,
        ),
        donate_argnums=donate,
        keep_unused=True,
    )

    def run(in_maps):
        concat_in = [
            np.concatenate([np.asarray(m[name]) for m in in_maps], axis=0)
            for name in in_names
        ]
        concat_zeros = [
            np.zeros((NCORES * s[0], *s[1:]), dt) for (s, dt) in zero_shapes
        ]
        out_arrs = sharded(*concat_in, *concat_zeros)
        return [
            {
                name: np.asarray(out_arrs[i]).reshape(NCORES, *zero_shapes[i][0])[c]
                for i, name in enumerate(out_names)
            }
            for c in range(NCORES)
        ]

    _RUNNER_CACHE = run
    return run


def _shard_inputs(mixture_latent, per_traj_weight, per_step_weight):
    in_maps = []
    for c in range(NCORES):
        sl = slice(c * BC, (c + 1) * BC)
        in_maps.append(
            {
                "mixture_latent": np.ascontiguousarray(
                    mixture_latent[sl], np.float32
                ).reshape(BC, T * M * 5),
                "per_traj_weight": np.ascontiguousarray(
                    per_traj_weight[sl], np.float32
                ),
                "per_step_weight": np.ascontiguousarray(
                    per_step_weight[sl], np.float32
                ),
            }
        )
    return in_maps


def _gather_outputs(results):
    probs_traj = np.concatenate([r["probs_traj"] for r in results], 0)
    loc = np.concatenate([r["loc"].reshape(BC, M, D) for r in results], 0)
    chol_full = np.concatenate(
        [r["chol_full"].reshape(BC, M, D, D) for r in results], 0
    )
    step_probs = np.concatenate(
        [r["step_probs"].reshape(BC, T, M) for r in results], 0
    )
    mu = np.concatenate([r["mu"].reshape(BC, T, M, 2) for r in results], 0)
    chol_step = np.concatenate(
        [r["chol_step"].reshape(BC, T, M, 2, 2) for r in results], 0
    )
    return (probs_traj, loc, chol_full, step_probs, mu, chol_step)


def kernel(mixture_latent, per_traj_weight, per_step_weight, _trace=False):
    in_maps = _shard_inputs(mixture_latent, per_traj_weight, per_step_weight)
    if _trace:
        nc = _get_nc()
        res = run_bass_kernel_spmd(nc, in_maps, list(range(NCORES)), trace=True)
        return _gather_outputs(res.results), res
    results = _get_runner()(in_maps)
    return _gather_outputs(results)


# revision 23
# speedup vs baseline: 3.6143x; 3.6143x over previous
"""Trainium2 Bass kernel for nn_MDN_MultivariateNormal (B=1024, T=30, M=20).

Pure data-parallel over batch: 8 NeuronCores x 128 batch rows each.
Batch rows map to the 128 SBUF partitions; everything else lives in the
free dimension, so all compute is single-pass elementwise/reduction work.

Outputs (per core shard, 128 rows):
  probs_traj [128,20], loc [128,20,60], chol_full [128,20,60,60],
  step_probs [128,30,20], mu [128,30,20,2], chol_step [128,30,20,2,2]

chol_full is 2x2-block-diagonal: nonzeros at flat offsets 122t (L11),
122t+60 (L21), 122t+61 (L22) within each 3600-elem [60,60] block.
We keep rotating SBUF buffers that are zeroed once, write only the three
strided diagonals per (m), and DMA the dense 14.4KB/partition block out.
"""

import sys

for _p in ("/root/.axon_site/_ro/trn_rl_repo", "/opt/trn_rl_repo"):
    if _p not in sys.path:
        sys.path.append(_p)

from contextlib import ExitStack

import numpy as np

import concourse.bacc as bacc
import concourse.bass as bass
import concourse.mybir as mybir
import concourse.tile as tile
from concourse.bass_utils import run_bass_kernel_spmd

F32 = mybir.dt.float32
AF = mybir.ActivationFunctionType
ALU = mybir.AluOpType
AX = mybir.AxisListType

B, T, M = 1024, 30, 20
NCORES = 8
BC = B // NCORES  # 128 batch rows per core == SBUF partitions
D = 2 * T  # 60
BLK = D * D  # 3600

# tunables
NBUF = 4  # rotating chol_full SBUF buffers (dense variant)
STEP_BCAST = True  # use stride-0 broadcast tensor_tensor for step softmax
SPARSE_CHOL = False  # measured 449us vs 124us dense: per-(m,t) diagonal
# runs are 4-8B descriptors and the stream becomes descriptor/RMW-bound,
# 3.6x worse than densely streaming the zero-filled blocks at line rate


def _diag_ap(base, extra_offset, step, count):
    """Strided free-dim AP into an SBUF tile: [128p][count elems, stride step]."""
    return bass.AP(
        tensor=base.tensor,
        offset=base.offset + extra_offset,
        ap=[list(base.ap[0]), [step, count]],
    )


def _build_body(ctx, tc, ins, outs):
    nc = tc.nc
    ml, ptw, psw = ins
    probs_traj, loc, chol_full, step_probs, mu, chol_step = outs

    pool = ctx.enter_context(tc.tile_pool(name="main", bufs=1))
    cpool = ctx.enter_context(tc.tile_pool(name="cf", bufs=1))

    # ---- load inputs (SP HWDGE ring; these precede the chol stream) ----
    X = pool.tile([BC, T, M, 5], F32, tag="X")
    nc.sync.dma_start(out=X[:], in_=ml.ap().rearrange("p (t m f) -> p t m f", t=T, m=M))
    PT = pool.tile([BC, M], F32, tag="PT")
    nc.sync.dma_start(out=PT[:], in_=ptw.ap())
    PS = pool.tile([BC, T, M], F32, tag="PS")
    nc.sync.dma_start(out=PS[:], in_=psw.ap().rearrange("p (t m) -> p t m", t=T))

    # ---- pre-zero the rotating chol buffers (zeros persist; only diagonals
    # are rewritten per m, so zero once per buffer) ----
    if not SPARSE_CHOL:
        cbufs = [
            cpool.tile([BC, BLK], F32, tag=f"cf{i}", name=f"cf{i}")
            for i in range(NBUF)
        ]
        for cb in cbufs:
            nc.vector.memset(cb[:], 0.0)

    # constant bias tiles for activation(func(scale*x + bias))
    bias_one = pool.tile([BC, 1], F32, tag="bias_one")
    nc.vector.memset(bias_one, 1.0)
    bias_gate = pool.tile([BC, 1], F32, tag="bias_gate")
    nc.vector.memset(bias_gate, -0.2)
    bias_eps = pool.tile([BC, 1], F32, tag="bias_eps")
    nc.vector.memset(bias_eps, 1e-6)

    # ---- per-component Cholesky params, in [BC, T, M] (input) layout ----
    # eps lands as the Sqrt activation's bias: keeps the whole chain on ACT
    # (no DVE round-trip on the critical path to the chol stream)
    SX = pool.tile([BC, T, M], F32, tag="SX")  # sqrt(exp(p2)+eps) == L11
    nc.scalar.activation(out=SX[:], in_=X[:, :, :, 2], func=AF.Exp)
    nc.scalar.activation(out=SX[:], in_=SX[:], func=AF.Sqrt, bias=bias_eps)

    SY = pool.tile([BC, T, M], F32, tag="SY")
    nc.scalar.activation(out=SY[:], in_=X[:, :, :, 3], func=AF.Exp)
    nc.scalar.activation(out=SY[:], in_=SY[:], func=AF.Sqrt, bias=bias_eps)

    RHO = pool.tile([BC, T, M], F32, tag="RHO")
    nc.scalar.activation(out=RHO[:], in_=X[:, :, :, 4], func=AF.Tanh)

    L21 = pool.tile([BC, T, M], F32, tag="L21")
    nc.vector.tensor_mul(out=L21[:], in0=RHO[:], in1=SY[:])

    L22 = pool.tile([BC, T, M], F32, tag="L22")
    nc.vector.tensor_mul(out=L22[:], in0=RHO[:], in1=RHO[:])
    nc.scalar.activation(out=L22[:], in_=L22[:], func=AF.Sqrt, scale=-1.0, bias=bias_one)
    nc.vector.tensor_mul(out=L22[:], in0=L22[:], in1=SY[:])

    # ---- loc [BC, M, T, 2] and mu [BC, T, M, 2] ----
    # issued on the sync HWDGE ring ahead of the chol stream: these only
    # need X, so their DMAs fill the queue while ACT/DVE are still
    # producing the Cholesky terms
    LOC = pool.tile([BC, M, T, 2], F32, tag="LOC")
    nc.vector.tensor_copy(out=LOC[:, :, :, 0], in_=X[:, :, :, 0].transpose([0, 2, 1]))
    nc.vector.tensor_copy(out=LOC[:, :, :, 1], in_=X[:, :, :, 1].transpose([0, 2, 1]))
    nc.sync.dma_start(out=loc.ap(), in_=LOC[:].rearrange("p a b c -> p (a b c)"))

    MUT = pool.tile([BC, T, M, 2], F32, tag="MUT")
    nc.vector.tensor_copy(out=MUT[:, :, :, 0], in_=X[:, :, :, 0])
    nc.vector.tensor_copy(out=MUT[:, :, :, 1], in_=X[:, :, :, 1])
    nc.sync.dma_start(out=mu.ap(), in_=MUT[:].rearrange("p a b c -> p (a b c)"))

    # ---- chol_step [BC, T, M, 2, 2] = [[L11,0],[L21,L22]] ----
    CS = pool.tile([BC, T, M, 4], F32, tag="CS")
    nc.vector.memset(CS[:, :, :, 1], 0.0)
    nc.vector.tensor_copy(out=CS[:, :, :, 0], in_=SX[:])
    nc.vector.tensor_copy(out=CS[:, :, :, 2], in_=L21[:])
    nc.vector.tensor_copy(out=CS[:, :, :, 3], in_=L22[:])
    nc.sync.dma_start(out=chol_step.ap(), in_=CS[:].rearrange("p a b c -> p (a b c)"))

    # ---- chol_full ----
    cf = chol_full.ap().rearrange("p (m k) -> p m k", m=M)
    if SPARSE_CHOL:
        # DRAM outputs arrive pre-zeroed (native run_bass_kernel_spmd zeros
        # them; the PJRT path donates zero buffers), so only the nonzero
        # diagonals need writing: per (m, t) the L11 scalar at 122t and the
        # [L21, L22] pair at 122t+60. DMA APs max out at 3 dims with a
        # contiguous last dim, so issue one DMA per (m, diagonal-kind).
        SL = pool.tile([BC, M, T, 2], F32, tag="SL")
        nc.vector.tensor_copy(out=SL[:, :, :, 0], in_=L21[:].transpose([0, 2, 1]))
        nc.vector.tensor_copy(out=SL[:, :, :, 1], in_=L22[:].transpose([0, 2, 1]))
        cfb = cf[:, 0, 0:1]  # AP anchor for manual strided APs
        pdim = list(cfb.ap[0])
        for m in range(M):
            l11_out = bass.AP(
                tensor=cfb.tensor,
                offset=cfb.offset + m * BLK,
                ap=[pdim, [2 * D + 2, T], [1, 1]],
            )
            nc.scalar.dma_start(
                out=l11_out, in_=SX[:, :, m].unsqueeze(2)
            )
            l2122_out = bass.AP(
                tensor=cfb.tensor,
                offset=cfb.offset + m * BLK + D,
                ap=[pdim, [2 * D + 2, T], [1, 2]],
            )
            nc.sync.dma_start(out=l2122_out, in_=SL[:, m, :, :])
    else:
        for m in range(M):
            cb = cbufs[m % NBUF]
            base = cb[:]
            # tensor_scalar_add (not tensor_copy): the DVE TensorCopy "TR"
            # encoding has a single sync-wait slot and walrus dies when Tile
            # needs two waits (producer RAW + DMA WAR) on one instruction.
            nc.vector.tensor_scalar_add(
                out=_diag_ap(base, 0, 2 * D + 2, T), in0=SX[:, :, m], scalar1=0.0
            )
            nc.vector.tensor_scalar_add(
                out=_diag_ap(base, D, 2 * D + 2, T), in0=L21[:, :, m], scalar1=0.0
            )
            nc.vector.tensor_scalar_add(
                out=_diag_ap(base, D + 1, 2 * D + 2, T), in0=L22[:, :, m], scalar1=0.0
            )
            nc.sync.dma_start(out=cf[:, m, :], in_=base)

    # ---- probs_traj: pruned softmax over M=20 ----
    E1 = pool.tile([BC, M], F32, tag="E1")
    nc.scalar.activation(out=E1[:], in_=PT[:], func=AF.Exp)
    s1 = pool.tile([BC, 1], F32, tag="s1")
    nc.vector.reduce_sum(out=s1[:], in_=E1[:], axis=AX.X)
    nc.vector.reciprocal(out=s1[:], in_=s1[:])
    nc.vector.tensor_scalar_mul(out=E1[:], in0=E1[:], scalar1=s1[:])  # p
    G1 = pool.tile([BC, M], F32, tag="G1")
    nc.scalar.activation(out=G1[:], in_=E1[:], func=AF.Sigmoid, scale=10.0, bias=bias_gate)
    nc.vector.tensor_mul(out=E1[:], in0=E1[:], in1=G1[:])  # p_tilde
    s2 = pool.tile([BC, 1], F32, tag="s2")
    nc.vector.reduce_sum(out=s2[:], in_=E1[:], axis=AX.X)
    nc.vector.tensor_scalar_add(out=s2[:], in0=s2[:], scalar1=1e-8)
    nc.vector.reciprocal(out=s2[:], in_=s2[:])
    nc.vector.tensor_scalar(
        out=E1[:], in0=E1[:], scalar1=s2[:], scalar2=1e-8, op0=ALU.mult, op1=ALU.add
    )  # p_new + eps
    s3 = pool.tile([BC, 1], F32, tag="s3")
    nc.vector.reduce_sum(out=s3[:], in_=E1[:], axis=AX.X)
    nc.vector.reciprocal(out=s3[:], in_=s3[:])
    nc.vector.tensor_scalar_mul(out=E1[:], in0=E1[:], scalar1=s3[:])
    nc.scalar.dma_start(out=probs_traj.ap(), in_=E1[:])

    # ---- step_probs: pruned softmax over M per (row, t) ----
    E2 = pool.tile([BC, T, M], F32, tag="E2")
    nc.scalar.activation(out=E2[:], in_=PS[:], func=AF.Exp)
    t1 = pool.tile([BC, T], F32, tag="t1")
    nc.vector.reduce_sum(out=t1[:], in_=E2[:], axis=AX.X)
    nc.vector.reciprocal(out=t1[:], in_=t1[:])

    def bmul(dst, src, vec):
        """dst[p,t,m] = src[p,t,m] * vec[p,t] (broadcast over m)."""
        if STEP_BCAST:
            nc.vector.tensor_tensor(
                out=dst, in0=src, in1=vec.broadcast_to([BC, T, M]), op=ALU.mult
            )
        else:
            for t in range(T):
                nc.vector.tensor_scalar_mul(
                    out=dst[:, t, :], in0=src[:, t, :], scalar1=vec[:, t : t + 1]
                )

    bmul(E2[:], E2[:], t1[:])  # p
    G2 = pool.tile([BC, T, M], F32, tag="G2")
    nc.scalar.activation(out=G2[:], in_=E2[:], func=AF.Sigmoid, scale=10.0, bias=bias_gate)
    nc.vector.tensor_mul(out=E2[:], in0=E2[:], in1=G2[:])  # p_tilde
    t2 = pool.tile([BC, T], F32, tag="t2")
    nc.vector.reduce_sum(out=t2[:], in_=E2[:], axis=AX.X)
    nc.vector.tensor_scalar_add(out=t2[:], in0=t2[:], scalar1=1e-8)
    nc.vector.reciprocal(out=t2[:], in_=t2[:])
    bmul(E2[:], E2[:], t2[:])
    nc.vector.tensor_scalar_add(out=E2[:], in0=E2[:], scalar1=1e-8)  # p_new + eps
    t3 = pool.tile([BC, T], F32, tag="t3")
    nc.vector.reduce_sum(out=t3[:], in_=E2[:], axis=AX.X)
    nc.vector.reciprocal(out=t3[:], in_=t3[:])
    bmul(E2[:], E2[:], t3[:])
    nc.scalar.dma_start(
        out=step_probs.ap(), in_=E2[:].rearrange("p a b -> p (a b)")
    )


def build_nc():
    # Bacc (not plain Bass): its compile() pass splits multi-wait sync_info
    # into EventSemaphore instructions -- TRN2 compute instructions accept
    # only one embedded wait and walrus rejects more.
    nc = bacc.Bacc("TRN2", target_bir_lowering=False, debug=False)
    ml = nc.dram_tensor("mixture_latent", [BC, T * M * 5], F32, kind="ExternalInput")
    ptw = nc.dram_tensor("per_traj_weight", [BC, M], F32, kind="ExternalInput")
    psw = nc.dram_tensor("per_step_weight", [BC, T * M], F32, kind="ExternalInput")
    o_probs = nc.dram_tensor("probs_traj", [BC, M], F32, kind="ExternalOutput")
    o_loc = nc.dram_tensor("loc", [BC, M * D], F32, kind="ExternalOutput")
    o_chol = nc.dram_tensor("chol_full", [BC, M * BLK], F32, kind="ExternalOutput")
    o_step = nc.dram_tensor("step_probs", [BC, T * M], F32, kind="ExternalOutput")
    o_mu = nc.dram_tensor("mu", [BC, T * M * 2], F32, kind="ExternalOutput")
    o_cs = nc.dram_tensor("chol_step", [BC, T * M * 4], F32, kind="ExternalOutput")
    with tile.TileContext(nc) as tc:
        with ExitStack() as ctx:
            _build_body(
                ctx,
                tc,
                (ml, ptw, psw),
                (o_probs, o_loc, o_chol, o_step, o_mu, o_cs),
            )
    nc.compile()
    return nc


_NC_CACHE = None


def _get_nc():
    global _NC_CACHE
    if _NC_CACHE is None:
        _NC_CACHE = build_nc()
    return _NC_CACHE


_RUNNER_CACHE = None


def _get_runner():
    """Build the jitted 8-core executable once and reuse it across calls.

    Mirrors concourse.bass2jax.run_bass_via_pjrt's multi-core path, but
    caches the jax.jit(shard_map(...)) callable so repeat kernel() calls
    skip re-lowering/XLA compile.
    """
    global _RUNNER_CACHE
    if _RUNNER_CACHE is not None:
        return _RUNNER_CACHE

    import jax
    from jax.experimental.shard_map import shard_map
    from jax.sharding import Mesh, PartitionSpec

    from concourse import bass2jax as b2j

    nc = _get_nc()
    b2j.install_neuronx_cc_hook()

    partition_name = (
        nc.partition_id_tensor.name if nc.partition_id_tensor else None
    )
    in_names: list[str] = []
    out_names: list[str] = []
    out_avals: list = []
    zero_shapes: list = []
    for alloc in nc.m.functions[0].allocations:
        if not isinstance(alloc, mybir.MemoryLocationSet):
            continue
        name = alloc.memorylocations[0].name
        if alloc.kind == "ExternalInput":
            if name != partition_name:
                in_names.append(name)
        elif alloc.kind == "ExternalOutput":
            shape = tuple(alloc.tensor_shape)
            dtype = mybir.dt.np(alloc.dtype)
            out_avals.append(jax.core.ShapedArray(shape, dtype))
            out_names.append(name)
            zero_shapes.append((shape, dtype))
    n_params = len(in_names)
    n_outs = len(out_names)
    all_names = in_names + out_names
    if partition_name is not None:
        all_names = all_names + [partition_name]

    def _body(*args):
        operands = list(args)
        if partition_name is not None:
            operands.append(b2j.partition_id_tensor())
        outs = b2j._bass_exec_p.bind(
            *operands,
            out_avals=tuple(out_avals),
            in_names=tuple(all_names),
            out_names=tuple(out_names),
            lowering_input_output_aliases=(),
            sim_require_finite=True,
            sim_require_nnan=True,
            nc=nc,
        )
        return tuple(outs)

    devices = jax.devices()[:NCORES]
    mesh = Mesh(np.asarray(devices), ("core",))
    donate = tuple(range(n_params, n_params + n_outs))
    sharded = jax.jit(
        shard_map(
            _body,
            mesh=mesh,
            in_specs=(PartitionSpec("core"),) * (n_params + n_outs),
            out_specs=(PartitionSpec("core"),) * n_outs,
            check_rep=False,
        ),
        donate_argnums=donate,
        keep_unused=True,
    )

    def run(in_maps):
        concat_in = [
            np.concatenate([np.asarray(m[name]) for m in in_maps], axis=0)
            for name in in_names
        ]
        concat_zeros = [
            np.zeros((NCORES * s[0], *s[1:]), dt) for (s, dt) in zero_shapes
        ]
        out_arrs = sharded(*concat_in, *concat_zeros)
        return [
            {
                name: np.asarray(out_arrs[i]).reshape(NCORES, *zero_shapes[i][0])[c]
                for i, name in enumerate(out_names)
            }
            for c in range(NCORES)
        ]

    _RUNNER_CACHE = run
    return run


def _shard_inputs(mixture_latent, per_traj_weight, per_step_weight):
    in_maps = []
    for c in range(NCORES):
        sl = slice(c * BC, (c + 1) * BC)
        in_maps.append(
            {
                "mixture_latent": np.ascontiguousarray(
                    mixture_latent[sl], np.float32
                ).reshape(BC, T * M * 5),
                "per_traj_weight": np.ascontiguousarray(
                    per_traj_weight[sl], np.float32
                ),
                "per_step_weight": np.ascontiguousarray(
                    per_step_weight[sl], np.float32
                ),
            }
        )
    return in_maps


def _gather_outputs(results):
    probs_traj = np.concatenate([r["probs_traj"] for r in results], 0)
    loc = np.concatenate([r["loc"].reshape(BC, M, D) for r in results], 0)
    chol_full = np.concatenate(
        [r["chol_full"].reshape(BC, M, D, D) for r in results], 0
    )
    step_probs = np.concatenate(
        [r["step_probs"].reshape(BC, T, M) for r in results], 0
    )
    mu = np.concatenate([r["mu"].reshape(BC, T, M, 2) for r in results], 0)
    chol_step = np.concatenate(
        [r["chol_step"].reshape(BC, T, M, 2, 2) for r in results], 0
    )
    return (probs_traj, loc, chol_full, step_probs, mu, chol_step)


def kernel(mixture_latent, per_traj_weight, per_step_weight, _trace=False):
    in_maps = _shard_inputs(mixture_latent, per_traj_weight, per_step_weight)
    if _trace:
        nc = _get_nc()
        res = run_bass_kernel_spmd(nc, in_maps, list(range(NCORES)), trace=True)
        return _gather_outputs(res.results), res
    try:
        results = _get_runner()(in_maps)
    except Exception:
        # fall back to the stock concourse SPMD runner (slower dispatch,
        # same NEFF) if the cached-jit fast path hits an environment quirk
        res = run_bass_kernel_spmd(_get_nc(), in_maps, list(range(NCORES)))
        results = res.results
    return _gather_outputs(results)


# revision 27
# speedup vs baseline: 3.7021x; 1.0243x over previous
"""Trainium2 Bass kernel for nn_MDN_MultivariateNormal (B=1024, T=30, M=20).

Pure data-parallel over batch: 8 NeuronCores x 128 batch rows each.
Batch rows map to the 128 SBUF partitions; everything else lives in the
free dimension, so all compute is single-pass elementwise/reduction work.

Outputs (per core shard, 128 rows):
  probs_traj [128,20], loc [128,20,60], chol_full [128,20,60,60],
  step_probs [128,30,20], mu [128,30,20,2], chol_step [128,30,20,2,2]

chol_full is 2x2-block-diagonal: nonzeros at flat offsets 122t (L11),
122t+60 (L21), 122t+61 (L22) within each 3600-elem [60,60] block.
We keep rotating SBUF buffers that are zeroed once, write only the three
strided diagonals per (m), and DMA the dense 14.4KB/partition block out.
"""

import sys

for _p in ("/root/.axon_site/_ro/trn_rl_repo", "/opt/trn_rl_repo"):
    if _p not in sys.path:
        sys.path.append(_p)

from contextlib import ExitStack

import numpy as np

import concourse.bacc as bacc
import concourse.bass as bass
import concourse.mybir as mybir
import concourse.tile as tile
from concourse.bass_utils import run_bass_kernel_spmd

F32 = mybir.dt.float32
AF = mybir.ActivationFunctionType
ALU = mybir.AluOpType
AX = mybir.AxisListType

B, T, M = 1024, 30, 20
NCORES = 8
BC = B // NCORES  # 128 batch rows per core == SBUF partitions
D = 2 * T  # 60
BLK = D * D  # 3600

# tunables
NBUF = 4  # rotating chol_full SBUF buffers (dense variant)
STEP_BCAST = True  # use stride-0 broadcast tensor_tensor for step softmax
SPARSE_CHOL = False  # measured 449us vs 124us dense: per-(m,t) diagonal
# runs are 4-8B descriptors and the stream becomes descriptor/RMW-bound,
# 3.6x worse than densely streaming the zero-filled blocks at line rate


def _diag_ap(base, extra_offset, step, count):
    """Strided free-dim AP into an SBUF tile: [128p][count elems, stride step]."""
    return bass.AP(
        tensor=base.tensor,
        offset=base.offset + extra_offset,
        ap=[list(base.ap[0]), [step, count]],
    )


def _build_body(ctx, tc, ins, outs):
    nc = tc.nc
    ml, ptw, psw = ins
    probs_traj, loc, chol_full, step_probs, mu, chol_step = outs

    pool = ctx.enter_context(tc.tile_pool(name="main", bufs=1))
    cpool = ctx.enter_context(tc.tile_pool(name="cf", bufs=1))

    # ---- load inputs (SP HWDGE ring; these precede the chol stream) ----
    X = pool.tile([BC, T, M, 5], F32, tag="X")
    nc.sync.dma_start(out=X[:], in_=ml.ap().rearrange("p (t m f) -> p t m f", t=T, m=M))
    PT = pool.tile([BC, M], F32, tag="PT")
    nc.sync.dma_start(out=PT[:], in_=ptw.ap())
    PS = pool.tile([BC, T, M], F32, tag="PS")
    nc.sync.dma_start(out=PS[:], in_=psw.ap().rearrange("p (t m) -> p t m", t=T))

    # ---- pre-zero the rotating chol buffers (zeros persist; only diagonals
    # are rewritten per m, so zero once per buffer). GPSIMD is otherwise
    # idle, so the big memsets never block the DVE/ACT critical path.
    if not SPARSE_CHOL:
        cbufs = [
            cpool.tile([BC, BLK], F32, tag=f"cf{i}", name=f"cf{i}")
            for i in range(NBUF)
        ]
        for cb in cbufs:
            nc.gpsimd.memset(cb[:], 0.0)

    # constant bias tiles for activation(func(scale*x + bias))
    bias_one = pool.tile([BC, 1], F32, tag="bias_one")
    nc.vector.memset(bias_one, 1.0)
    bias_gate = pool.tile([BC, 1], F32, tag="bias_gate")
    nc.vector.memset(bias_gate, -0.2)
    bias_eps = pool.tile([BC, 1], F32, tag="bias_eps")
    nc.vector.memset(bias_eps, 1e-6)

    # ---- per-component Cholesky params, in [BC, T, M] (input) layout ----
    # eps lands as the Sqrt activation's bias: keeps the whole chain on ACT
    # (no DVE round-trip on the critical path to the chol stream).
    # Computed in m-chunks so the first chol_full DMAs launch after ~1/CH
    # of the transcendental work instead of all of it.
    SX = pool.tile([BC, T, M], F32, tag="SX")  # sqrt(exp(p2)+eps) == L11
    SY = pool.tile([BC, T, M], F32, tag="SY")
    RHO = pool.tile([BC, T, M], F32, tag="RHO")
    L21 = pool.tile([BC, T, M], F32, tag="L21")
    L22 = pool.tile([BC, T, M], F32, tag="L22")

    def l_chain(sl):
        nc.scalar.activation(out=SX[:, :, sl], in_=X[:, :, sl, 2], func=AF.Exp)
        nc.scalar.activation(
            out=SX[:, :, sl], in_=SX[:, :, sl], func=AF.Sqrt, bias=bias_eps
        )
        nc.scalar.activation(out=SY[:, :, sl], in_=X[:, :, sl, 3], func=AF.Exp)
        nc.scalar.activation(
            out=SY[:, :, sl], in_=SY[:, :, sl], func=AF.Sqrt, bias=bias_eps
        )
        nc.scalar.activation(out=RHO[:, :, sl], in_=X[:, :, sl, 4], func=AF.Tanh)
        nc.vector.tensor_mul(out=L21[:, :, sl], in0=RHO[:, :, sl], in1=SY[:, :, sl])
        nc.vector.tensor_mul(out=L22[:, :, sl], in0=RHO[:, :, sl], in1=RHO[:, :, sl])
        nc.scalar.activation(
            out=L22[:, :, sl], in_=L22[:, :, sl], func=AF.Sqrt, scale=-1.0,
            bias=bias_one,
        )
        nc.vector.tensor_mul(out=L22[:, :, sl], in0=L22[:, :, sl], in1=SY[:, :, sl])

    # ---- chol_full ----
    cf = chol_full.ap().rearrange("p (m k) -> p m k", m=M)
    if SPARSE_CHOL:
        l_chain(slice(0, M))
        # DRAM outputs arrive pre-zeroed (native run_bass_kernel_spmd zeros
        # them; the PJRT path donates zero buffers), so only the nonzero
        # diagonals need writing: per (m, t) the L11 scalar at 122t and the
        # [L21, L22] pair at 122t+60. DMA APs max out at 3 dims with a
        # contiguous last dim, so issue one DMA per (m, diagonal-kind).
        SL = pool.tile([BC, M, T, 2], F32, tag="SL")
        nc.vector.tensor_copy(out=SL[:, :, :, 0], in_=L21[:].transpose([0, 2, 1]))
        nc.vector.tensor_copy(out=SL[:, :, :, 1], in_=L22[:].transpose([0, 2, 1]))
        cfb = cf[:, 0, 0:1]  # AP anchor for manual strided APs
        pdim = list(cfb.ap[0])
        for m in range(M):
            l11_out = bass.AP(
                tensor=cfb.tensor,
                offset=cfb.offset + m * BLK,
                ap=[pdim, [2 * D + 2, T], [1, 1]],
            )
            nc.scalar.dma_start(
                out=l11_out, in_=SX[:, :, m].unsqueeze(2)
            )
            l2122_out = bass.AP(
                tensor=cfb.tensor,
                offset=cfb.offset + m * BLK + D,
                ap=[pdim, [2 * D + 2, T], [1, 2]],
            )
            nc.sync.dma_start(out=l2122_out, in_=SL[:, m, :, :])
    else:
        CH = 5  # m's per compute chunk
        for m0 in range(0, M, CH):
            l_chain(slice(m0, m0 + CH))
            for m in range(m0, m0 + CH):
                cb = cbufs[m % NBUF]
                base = cb[:]
                nc.vector.tensor_scalar_add(
                    out=_diag_ap(base, 0, 2 * D + 2, T), in0=SX[:, :, m], scalar1=0.0
                )
                nc.vector.tensor_scalar_add(
                    out=_diag_ap(base, D, 2 * D + 2, T), in0=L21[:, :, m], scalar1=0.0
                )
                nc.vector.tensor_scalar_add(
                    out=_diag_ap(base, D + 1, 2 * D + 2, T),
                    in0=L22[:, :, m],
                    scalar1=0.0,
                )
                nc.sync.dma_start(out=cf[:, m, :], in_=base)

    # ---- loc [BC, M, T, 2] and mu [BC, T, M, 2] (scalar HWDGE ring) ----
    LOC = pool.tile([BC, M, T, 2], F32, tag="LOC")
    nc.vector.tensor_copy(out=LOC[:, :, :, 0], in_=X[:, :, :, 0].transpose([0, 2, 1]))
    nc.vector.tensor_copy(out=LOC[:, :, :, 1], in_=X[:, :, :, 1].transpose([0, 2, 1]))
    nc.scalar.dma_start(out=loc.ap(), in_=LOC[:].rearrange("p a b c -> p (a b c)"))

    MUT = pool.tile([BC, T, M, 2], F32, tag="MUT")
    nc.vector.tensor_copy(out=MUT[:, :, :, 0], in_=X[:, :, :, 0])
    nc.vector.tensor_copy(out=MUT[:, :, :, 1], in_=X[:, :, :, 1])
    nc.scalar.dma_start(out=mu.ap(), in_=MUT[:].rearrange("p a b c -> p (a b c)"))

    # ---- chol_step [BC, T, M, 2, 2] = [[L11,0],[L21,L22]] ----
    CS = pool.tile([BC, T, M, 4], F32, tag="CS")
    nc.vector.memset(CS[:, :, :, 1], 0.0)
    nc.vector.tensor_copy(out=CS[:, :, :, 0], in_=SX[:])
    nc.vector.tensor_copy(out=CS[:, :, :, 2], in_=L21[:])
    nc.vector.tensor_copy(out=CS[:, :, :, 3], in_=L22[:])
    nc.scalar.dma_start(out=chol_step.ap(), in_=CS[:].rearrange("p a b c -> p (a b c)"))

    # ---- probs_traj: pruned softmax over M=20 ----
    E1 = pool.tile([BC, M], F32, tag="E1")
    nc.scalar.activation(out=E1[:], in_=PT[:], func=AF.Exp)
    s1 = pool.tile([BC, 1], F32, tag="s1")
    nc.vector.reduce_sum(out=s1[:], in_=E1[:], axis=AX.X)
    nc.vector.reciprocal(out=s1[:], in_=s1[:])
    nc.vector.tensor_scalar_mul(out=E1[:], in0=E1[:], scalar1=s1[:])  # p
    G1 = pool.tile([BC, M], F32, tag="G1")
    nc.scalar.activation(out=G1[:], in_=E1[:], func=AF.Sigmoid, scale=10.0, bias=bias_gate)
    nc.vector.tensor_mul(out=E1[:], in0=E1[:], in1=G1[:])  # p_tilde
    s2 = pool.tile([BC, 1], F32, tag="s2")
    nc.vector.reduce_sum(out=s2[:], in_=E1[:], axis=AX.X)
    nc.vector.tensor_scalar_add(out=s2[:], in0=s2[:], scalar1=1e-8)
    nc.vector.reciprocal(out=s2[:], in_=s2[:])
    nc.vector.tensor_scalar(
        out=E1[:], in0=E1[:], scalar1=s2[:], scalar2=1e-8, op0=ALU.mult, op1=ALU.add
    )  # p_new + eps
    s3 = pool.tile([BC, 1], F32, tag="s3")
    nc.vector.reduce_sum(out=s3[:], in_=E1[:], axis=AX.X)
    nc.vector.reciprocal(out=s3[:], in_=s3[:])
    nc.vector.tensor_scalar_mul(out=E1[:], in0=E1[:], scalar1=s3[:])
    nc.scalar.dma_start(out=probs_traj.ap(), in_=E1[:])

    # ---- step_probs: pruned softmax over M per (row, t) ----
    E2 = pool.tile([BC, T, M], F32, tag="E2")
    nc.scalar.activation(out=E2[:], in_=PS[:], func=AF.Exp)
    t1 = pool.tile([BC, T], F32, tag="t1")
    nc.vector.reduce_sum(out=t1[:], in_=E2[:], axis=AX.X)
    nc.vector.reciprocal(out=t1[:], in_=t1[:])

    def bmul(dst, src, vec):
        """dst[p,t,m] = src[p,t,m] * vec[p,t] (broadcast over m)."""
        if STEP_BCAST:
            nc.vector.tensor_tensor(
                out=dst, in0=src, in1=vec.broadcast_to([BC, T, M]), op=ALU.mult
            )
        else:
            for t in range(T):
                nc.vector.tensor_scalar_mul(
                    out=dst[:, t, :], in0=src[:, t, :], scalar1=vec[:, t : t + 1]
                )

    bmul(E2[:], E2[:], t1[:])  # p
    G2 = pool.tile([BC, T, M], F32, tag="G2")
    nc.scalar.activation(out=G2[:], in_=E2[:], func=AF.Sigmoid, scale=10.0, bias=bias_gate)
    nc.vector.tensor_mul(out=E2[:], in0=E2[:], in1=G2[:])  # p_tilde
    t2 = pool.tile([BC, T], F32, tag="t2")
    nc.vector.reduce_sum(out=t2[:], in_=E2[:], axis=AX.X)
    nc.vector.tensor_scalar_add(out=t2[:], in0=t2[:], scalar1=1e-8)
    nc.vector.reciprocal(out=t2[:], in_=t2[:])
    bmul(E2[:], E2[:], t2[:])
    nc.vector.tensor_scalar_add(out=E2[:], in0=E2[:], scalar1=1e-8)  # p_new + eps
    t3 = pool.tile([BC, T], F32, tag="t3")
    nc.vector.reduce_sum(out=t3[:], in_=E2[:], axis=AX.X)
    nc.vector.reciprocal(out=t3[:], in_=t3[:])
    bmul(E2[:], E2[:], t3[:])
    nc.scalar.dma_start(
        out=step_probs.ap(), in_=E2[:].rearrange("p a b -> p (a b)")
    )


def build_nc():
    # Bacc (not plain Bass): its compile() pass splits multi-wait sync_info
    # into EventSemaphore instructions -- TRN2 compute instructions accept
    # only one embedded wait and walrus rejects more.
    nc = bacc.Bacc("TRN2", target_bir_lowering=False, debug=False)
    ml = nc.dram_tensor("mixture_latent", [BC, T * M * 5], F32, kind="ExternalInput")
    ptw = nc.dram_tensor("per_traj_weight", [BC, M], F32, kind="ExternalInput")
    psw = nc.dram_tensor("per_step_weight", [BC, T * M], F32, kind="ExternalInput")
    o_probs = nc.dram_tensor("probs_traj", [BC, M], F32, kind="ExternalOutput")
    o_loc = nc.dram_tensor("loc", [BC, M * D], F32, kind="ExternalOutput")
    o_chol = nc.dram_tensor("chol_full", [BC, M * BLK], F32, kind="ExternalOutput")
    o_step = nc.dram_tensor("step_probs", [BC, T * M], F32, kind="ExternalOutput")
    o_mu = nc.dram_tensor("mu", [BC, T * M * 2], F32, kind="ExternalOutput")
    o_cs = nc.dram_tensor("chol_step", [BC, T * M * 4], F32, kind="ExternalOutput")
    with tile.TileContext(nc) as tc:
        with ExitStack() as ctx:
            _build_body(
                ctx,
                tc,
                (ml, ptw, psw),
                (o_probs, o_loc, o_chol, o_step, o_mu, o_cs),
            )
    nc.compile()
    return nc


_NC_CACHE = None


def _get_nc():
    global _NC_CACHE
    if _NC_CACHE is None:
        _NC_CACHE = build_nc()
    return _NC_CACHE


_RUNNER_CACHE = None


def _get_runner():
    """Build the jitted 8-core executable once and reuse it across calls.

    Mirrors concourse.bass2jax.run_bass_via_pjrt's multi-core path, but
    caches the jax.jit(shard_map(...)) callable so repeat kernel() calls
    skip re-lowering/XLA compile.
    """
    global _RUNNER_CACHE
    if _RUNNER_CACHE is not None:
        return _RUNNER_CACHE

    import jax
    from jax.experimental.shard_map import shard_map
    from jax.sharding import Mesh, PartitionSpec

    from concourse import bass2jax as b2j

    nc = _get_nc()
    b2j.install_neuronx_cc_hook()

    partition_name = (
        nc.partition_id_tensor.name if nc.partition_id_tensor else None
    )
    in_names: list[str] = []
    out_names: list[str] = []
    out_avals: list = []
    zero_shapes: list = []
    for alloc in nc.m.functions[0].allocations:
        if not isinstance(alloc, mybir.MemoryLocationSet):
            continue
        name = alloc.memorylocations[0].name
        if alloc.kind == "ExternalInput":
            if name != partition_name:
                in_names.append(name)
        elif alloc.kind == "ExternalOutput":
            shape = tuple(alloc.tensor_shape)
            dtype = mybir.dt.np(alloc.dtype)
            out_avals.append(jax.core.ShapedArray(shape, dtype))
            out_names.append(name)
            zero_shapes.append((shape, dtype))
    n_params = len(in_names)
    n_outs = len(out_names)
    all_names = in_names + out_names
    if partition_name is not None:
        all_names = all_names + [partition_name]

    def _body(*args):
        operands = list(args)
        if partition_name is not None:
            operands.append(b2j.partition_id_tensor())
        outs = b2j._bass_exec_p.bind(
            *operands,
            out_avals=tuple(out_avals),
            in_names=tuple(all_names),
            out_names=tuple(out_names),
            lowering_input_output_aliases=(),
            sim_require_finite=True,
            sim_require_nnan=True,
            nc=nc,
        )
        return tuple(outs)

    devices = jax.devices()[:NCORES]
    mesh = Mesh(np.asarray(devices), ("core",))
    donate = tuple(range(n_params, n_params + n_outs))
    sharded = jax.jit(
        shard_map(
            _body,
            mesh=mesh,
            in_specs=(PartitionSpec("core"),) * (n_params + n_outs),
            out_specs=(PartitionSpec("core"),) * n_outs,
            check_rep=False,
        ),
        donate_argnums=donate,
        keep_unused=True,
    )

    def run(in_maps):
        concat_in = [
            np.concatenate([np.asarray(m[name]) for m in in_maps], axis=0)
            for name in in_names
        ]
        concat_zeros = [
            np.zeros((NCORES * s[0], *s[1:]), dt) for (s, dt) in zero_shapes
        ]
        out_arrs = sharded(*concat_in, *concat_zeros)
        return [
            {
                name: np.asarray(out_arrs[i]).reshape(NCORES, *zero_shapes[i][0])[c]
                for i, name in enumerate(out_names)
            }
            for c in range(NCORES)
        ]

    _RUNNER_CACHE = run
    return run


def _shard_inputs(mixture_latent, per_traj_weight, per_step_weight):
    in_maps = []
    for c in range(NCORES):
        sl = slice(c * BC, (c + 1) * BC)
        in_maps.append(
            {
                "mixture_latent": np.ascontiguousarray(
                    mixture_latent[sl], np.float32
                ).reshape(BC, T * M * 5),
                "per_traj_weight": np.ascontiguousarray(
                    per_traj_weight[sl], np.float32
                ),
                "per_step_weight": np.ascontiguousarray(
                    per_step_weight[sl], np.float32
                ),
            }
        )
    return in_maps


def _gather_outputs(results):
    probs_traj = np.concatenate([r["probs_traj"] for r in results], 0)
    loc = np.concatenate([r["loc"].reshape(BC, M, D) for r in results], 0)
    chol_full = np.concatenate(
        [r["chol_full"].reshape(BC, M, D, D) for r in results], 0
    )
    step_probs = np.concatenate(
        [r["step_probs"].reshape(BC, T, M) for r in results], 0
    )
    mu = np.concatenate([r["mu"].reshape(BC, T, M, 2) for r in results], 0)
    chol_step = np.concatenate(
        [r["chol_step"].reshape(BC, T, M, 2, 2) for r in results], 0
    )
    return (probs_traj, loc, chol_full, step_probs, mu, chol_step)


def kernel(mixture_latent, per_traj_weight, per_step_weight, _trace=False):
    in_maps = _shard_inputs(mixture_latent, per_traj_weight, per_step_weight)
    if _trace:
        nc = _get_nc()
        res = run_bass_kernel_spmd(nc, in_maps, list(range(NCORES)), trace=True)
        return _gather_outputs(res.results), res
    try:
        results = _get_runner()(in_maps)
    except Exception:
        # fall back to the stock concourse SPMD runner (slower dispatch,
        # same NEFF) if the cached-jit fast path hits an environment quirk
        res = run_bass_kernel_spmd(_get_nc(), in_maps, list(range(NCORES)))
        results = res.results
    return _gather_outputs(results)
